# revision 1
# baseline (speedup 1.0000x reference)
"""Local (sliding-window causal) attention on 8 Trainium2 NeuronCores.

Problem: B=1, S=4096, D=1024, H=16 heads (hd=64), WINDOW=256.
Sharding: tensor-parallel over heads -- 2 heads per core. Each core computes
q/k/v projections for its 2 heads, windowed softmax attention, and its
partial contribution o_c @ Wo_c. The host sums the 8 partials and adds the
bias terms.

Math notes:
 - score uses (q + bq) . (k + bk); the q.bk and bq.bk terms are constant per
   query row so they drop under softmax -> bk is dropped, bq folded into q.
 - v bias: o = p @ (v + bv) = p @ v + bv (softmax rows sum to 1), so the bv
   contribution to the output is the constant row bv @ Wo, added on host.
 - All matmuls run in float32r (tf32-like, full PE speed at N>=256),
   softmax statistics in f32.

Layouts on device (per core):
 - xT     [1024, 4096]   x transposed (host-provided), streamed in 512-col chunks
 - qT,kT  [128, S(+pad)]  head-dim on partitions (2 heads x 64), seq on free
 - v      [128, 34*128]  34 key blocks of [128 keys, 128 hd2]; first 2 blocks zero
 - oT     [128, 4096]    attention output transposed
 - y      [4096, 1024]   partial output (= oT.T @ Wo_c)

Sliding window: queries processed in super-blocks of 256 with a 512-key
padded window [sb*256-256, sb*256+256). Each 128-query half sees 384
contiguous keys of that window; bands masked additively before exp.
"""

import numpy as np

import concourse.bass as bass
import concourse.tile as tile
from concourse import bacc, mybir
from concourse.bass_utils import run_bass_kernel_spmd

# Problem constants (hardcoded per contract -- kernel.py must be self-contained)
S = 4096
D = 1024
H = 16
HD = 64
WINDOW = 256
N_CORES = 8
HPC = H // N_CORES          # heads per core = 2
DH = HPC * HD               # per-core head dims = 128
PAD = 256                   # zero left-padding of keys
SP = S + PAD                # padded key length = 4352
NEG = -1e9

F32 = mybir.dt.float32
F32R = mybir.dt.float32r

N_SB = S // 256             # 16 query super-blocks
N_QB = S // 128             # 32 query blocks
N_T = S // 512              # 8 projection seq chunks
KC = D // 128               # 8 contraction chunks


def _make_masks():
    """Additive masks [128, 384] for one 128-query half of a super-block.

    Query row qi (0..127) may see window-local columns jcol with
    qi+1 <= jcol <= qi+256 (same for both halves). For the first
    super-block, keys left of the sequence start are also masked:
    half 0 requires jcol >= 256, half 1 requires jcol >= 128.
    """
    qi = np.arange(128)[:, None]
    j = np.arange(384)[None, :]
    base = (j >= qi + 1) & (j <= qi + 256)
    m_g = np.where(base, 0.0, NEG).astype(np.float32)
    m0_h0 = np.where(base & (j >= 256), 0.0, NEG).astype(np.float32)
    m0_h1 = np.where(base & (j >= 128), 0.0, NEG).astype(np.float32)
    return m_g, m0_h0, m0_h1


def build_kernel():
    nc = bacc.Bacc()

    xT = nc.dram_tensor("xT", [D, S], F32, kind="ExternalInput")
    wq = nc.dram_tensor("wq", [D, DH], F32, kind="ExternalInput")
    wk = nc.dram_tensor("wk", [D, DH], F32, kind="ExternalInput")
    wv = nc.dram_tensor("wv", [D, DH], F32, kind="ExternalInput")
    bq = nc.dram_tensor("bq", [DH], F32, kind="ExternalInput")
    wo = nc.dram_tensor("wo", [DH, D], F32, kind="ExternalInput")
    y = nc.dram_tensor("y", [S, D], F32, kind="ExternalOutput")

    m_g, m0_h0, m0_h1 = _make_masks()
    mask_g_d = nc.inline_tensor(m_g, name="mask_g")
    mask0_d = [nc.inline_tensor(m0_h0, name="mask0_h0"),
               nc.inline_tensor(m0_h1, name="mask0_h1")]
    ident_d = nc.inline_tensor(np.eye(128, dtype=np.float32), name="ident")

    scale = 1.0 / float(np.sqrt(HD))

    with tile.TileContext(nc) as tc:
        with (
            tc.tile_pool(name="consts", bufs=1) as consts,
            tc.tile_pool(name="persist", bufs=1) as persist,
            tc.tile_pool(name="xstream", bufs=2) as xstream,
            tc.tile_pool(name="work", bufs=3) as work,
            tc.tile_pool(name="ppool", bufs=2) as ppool,
            tc.tile_pool(name="proj_ps", bufs=2, space="PSUM") as proj_ps,
            tc.tile_pool(name="attn_ps", bufs=3, space="PSUM") as attn_ps,
            tc.tile_pool(name="ot_ps", bufs=1, space="PSUM") as ot_ps,
            tc.tile_pool(name="y_ps", bufs=1, space="PSUM") as y_ps,
        ):
            # ---- constants to SBUF ----
            wq_t = consts.tile([128, KC * DH], F32R, name="wq_t")
            wk_t = consts.tile([128, KC * DH], F32R, name="wk_t")
            wv_t = consts.tile([128, KC * DH], F32R, name="wv_t")
            for (t, d) in ((wq_t, wq), (wk_t, wk), (wv_t, wv)):
                d3 = d.ap().rearrange("(c p) m -> p c m", p=128)
                for c in range(KC):
                    nc.sync.dma_start(t[:, c * DH:(c + 1) * DH], d3[:, c].bitcast(F32R))
            wo_t = consts.tile([DH, D], F32R, name="wo_t")
            nc.sync.dma_start(wo_t, wo.ap().bitcast(F32R))

            mask_g = consts.tile([128, 384], F32, name="mask_g")
            nc.sync.dma_start(mask_g, mask_g_d.ap())
            mask0 = []
            for u in range(2):
                mt = consts.tile([128, 384], F32, name=f"mask0_{u}", tag=f"mask0_{u}")
                nc.sync.dma_start(mt, mask0_d[u].ap())
                mask0.append(mt)
            ident = consts.tile([128, 128], F32R, name="ident")
            nc.sync.dma_start(ident, ident_d.ap().bitcast(F32R))

            bq_t = consts.tile([DH, 1], F32, name="bq_t")
            nc.sync.dma_start(bq_t, bq.ap().rearrange("(p o) -> p o", o=1))
            bqs = consts.tile([DH, 1], F32, name="bqs")
            nc.vector.tensor_scalar_mul(bqs, bq_t, scale)

            # ---- persistent activations ----
            qT = persist.tile([128, S], F32R, name="qT")
            kT = persist.tile([128, SP], F32R, name="kT")
            vv = persist.tile([128, (SP // 128) * 128], F32R, name="vv")
            oT = persist.tile([128, S], F32R, name="oT")
            nc.vector.memset(kT[:, 0:PAD].bitcast(F32), 0.0)
            nc.vector.memset(vv[:, 0:PAD].bitcast(F32), 0.0)

            # ---- projections ----
            for t in range(N_T):
                sl = slice(t * 512, (t + 1) * 512)
                xt = xstream.tile([128, KC, 512], F32R, name="xt")
                for c in range(KC):
                    nc.sync.dma_start(
                        xt[:, c], xT.ap()[c * 128:(c + 1) * 128, sl].bitcast(F32R))

                qps = proj_ps.tile([128, 512], F32, name="pps", tag="pps")
                for c in range(KC):
                    nc.tensor.matmul(qps, wq_t[:, c * DH:(c + 1) * DH], xt[:, c],
                                     start=(c == 0), stop=(c == KC - 1))
                nc.scalar.activation(qT[:, sl], qps,
                                     mybir.ActivationFunctionType.Identity,
                                     bias=bqs, scale=scale)

                kps = proj_ps.tile([128, 512], F32, name="kps", tag="pps")
                for c in range(KC):
                    nc.tensor.matmul(kps, wk_t[:, c * DH:(c + 1) * DH], xt[:, c],
                                     start=(c == 0), stop=(c == KC - 1))
                nc.scalar.copy(kT[:, PAD + t * 512:PAD + (t + 1) * 512], kps)

                vps = proj_ps.tile([128, 512], F32, name="vps", tag="pps")
                for c in range(KC):
                    nc.tensor.matmul(vps, wv_t[:, c * DH:(c + 1) * DH], xt[:, c],
                                     start=(c == 0), stop=(c == KC - 1))
                vt = work.tile([128, 512], F32R, name="vt", tag="vt")
                nc.scalar.copy(vt, vps)
                # transpose [hd2, seq] -> [seq, hd2] blocks into vv
                for b in range(4):
                    tb = proj_ps.tile([128, 128], F32R, name="tb", tag="pps")
                    nc.tensor.transpose(tb, vt[:, b * 128:(b + 1) * 128], ident)
                    blk = 2 + t * 4 + b
                    nc.scalar.copy(vv[:, blk * 128:(blk + 1) * 128], tb)

            # ---- attention ----
            for sb in range(N_SB):
                pT = []
                for h in range(2):
                    pt = ppool.tile([128, 4, 2, 128], F32R, name=f"pT{h}",
                                    tag=f"pT{h}")
                    nc.gpsimd.memset(pt[:, 3, 0].bitcast(F32), 0.0)
                    nc.gpsimd.memset(pt[:, 0, 1].bitcast(F32), 0.0)
                    pT.append(pt)
                for u in range(2):
                    qb = sb * 2 + u
                    qsl = slice(qb * 128, (qb + 1) * 128)
                    wsl = slice(sb * 256 + u * 128, sb * 256 + u * 128 + 384)
                    mask = mask0[u] if sb == 0 else mask_g
                    for h in range(2):
                        hsl = slice(h * 64, (h + 1) * 64)
                        sps = attn_ps.tile([128, 384], F32, name="sps", tag="aps")
                        nc.tensor.matmul(sps, qT[hsl, qsl], kT[hsl, wsl],
                                         start=True, stop=True)
                        sm = work.tile([128, 384], F32, name="sm", tag="sm")
                        nc.vector.scalar_tensor_tensor(
                            sm, sps, 1.0, mask,
                            op0=mybir.AluOpType.mult, op1=mybir.AluOpType.add)
                        p = work.tile([128, 384], F32R, name="p", tag="p")
                        rs = work.tile([128, 1], F32, name="rs", tag="rs")
                        nc.scalar.activation(p, sm,
                                             mybir.ActivationFunctionType.Exp,
                                             accum_out=rs)
                        rc = work.tile([128, 1], F32, name="rc", tag="rc")
                        nc.vector.reciprocal(rc, rs)
                        pn = work.tile([128, 384], F32R, name="pn", tag="pn")
                        nc.vector.tensor_scalar_mul(pn, p, rc)
                        tps = attn_ps.tile([128, 384], F32R, name="tps", tag="aps")
                        for kb3 in range(3):
                            nc.tensor.transpose(tps[:, kb3 * 128:(kb3 + 1) * 128],
                                                pn[:, kb3 * 128:(kb3 + 1) * 128],
                                                ident)
                        # window-local key blocks u..u+2, half u
                        nc.vector.tensor_copy(pT[h][:, u:u + 3, u], tps)

                ot = ot_ps.tile([64, 512], F32, name="ot")
                for h in range(2):
                    for kb in range(4):
                        blk = sb * 2 + kb
                        vsl = vv[:, blk * 128 + h * 64: blk * 128 + h * 64 + 64]
                        nc.tensor.matmul(ot[:, h * 256:h * 256 + 256], vsl,
                                         pT[h][:, kb],
                                         start=(kb == 0), stop=(kb == 3))
                ssl = slice(sb * 256, (sb + 1) * 256)
                nc.scalar.copy(oT[0:64, ssl], ot[:, 0:256])
                nc.scalar.copy(oT[64:128, ssl], ot[:, 256:512])

            # ---- output projection (partial: this core's heads only) ----
            for qb in range(N_QB):
                yps = y_ps.tile([128, 1024], F32, name="yps")
                for nch in range(2):
                    nc.tensor.matmul(yps[:, nch * 512:(nch + 1) * 512],
                                     oT[:, qb * 128:(qb + 1) * 128],
                                     wo_t[:, nch * 512:(nch + 1) * 512],
                                     start=True, stop=True)
                ysb = work.tile([128, 1024], F32, name="ysb", tag="ysb")
                nc.vector.tensor_copy(ysb, yps)
                nc.sync.dma_start(y.ap()[qb * 128:(qb + 1) * 128, :], ysb)

    if not nc.is_finalized():
        nc.finalize()
    return nc


_NC_CACHE = None


def kernel(x, Wq, bq, Wk, bk, Wv, bv, Wo, bo, **_kw):
    global _NC_CACHE
    x = np.asarray(x, dtype=np.float32)
    Wq = np.asarray(Wq, dtype=np.float32)
    Wk = np.asarray(Wk, dtype=np.float32)
    Wv = np.asarray(Wv, dtype=np.float32)
    Wo = np.asarray(Wo, dtype=np.float32)
    bq = np.asarray(bq, dtype=np.float32)
    bv = np.asarray(bv, dtype=np.float32)
    bo = np.asarray(bo, dtype=np.float32)

    B = x.shape[0]
    assert x.shape == (B, S, D) and B == 1

    xT = np.ascontiguousarray(x[0].T)

    in_maps = []
    for c in range(N_CORES):
        csl = slice(c * DH, (c + 1) * DH)
        in_maps.append({
            "xT": xT,
            "wq": np.ascontiguousarray(Wq[:, csl]),
            "wk": np.ascontiguousarray(Wk[:, csl]),
            "wv": np.ascontiguousarray(Wv[:, csl]),
            "bq": np.ascontiguousarray(bq[csl]),
            "wo": np.ascontiguousarray(Wo[csl, :]),
        })

    if _NC_CACHE is None:
        _NC_CACHE = build_kernel()
    res = run_bass_kernel_spmd(_NC_CACHE, in_maps, core_ids=list(range(N_CORES)))

    out = np.zeros((S, D), dtype=np.float32)
    for c in range(N_CORES):
        out += res.results[c]["y"]
    # host-side bias terms: bo plus the bv @ Wo constant row (see header)
    out += (bv @ Wo + bo)[None, :]
    return out.reshape(1, S, D)



# revision 23
# speedup vs baseline: 1.6335x; 1.6335x over previous
"""Local (sliding-window causal) attention on 8 Trainium2 NeuronCores. v2.

Problem: B=1, S=4096, D=1024, H=16 heads (hd=64), WINDOW=256.
Sharding: tensor-parallel over heads -- 2 heads per core. Each core computes
q/k/v projections for its 2 heads, windowed softmax attention, and its
partial contribution o_c @ Wo_c. The host sums the 8 partials and adds the
bias terms.

v2 design (vs v1): all matmuls in bf16 (FWL weight loads, full PE rate at
any N), scores computed TRANSPOSED (keys on partitions) so the probability
matrix feeds the PV matmul directly from SBUF with no PE transposes and no
PSUM->SBUF copies. Row sums come for free from a ones-column appended to V.
Softmax normalization is applied to the attention output oT via a
partition-broadcast reciprocal, then y = oT.T @ Wo is DMA'd to HBM straight
from PSUM. Projections, attention, and the output GEMM are interleaved so
the PE never idles (keeps the HAM clock-gate warm).

Math notes:
 - score uses (q + bq) . k; bk drops under softmax (constant per query),
   bq is folded into qT at the projection PSUM evacuation (with the 1/8
   softmax scale). bv contributes the constant row bv @ Wo, added on host.
 - windowed mask: per 128-query block the 384-key window splits into three
   aligned 128-key chunks: chunk c0 (strict upper tri), c1 (fully allowed,
   no mask op), c2 (lower incl tri). Masks are additive (0/-1e9), applied
   in PSUM before exp. Chunks are laid out [c0, c2, c1] in PSUM so the two
   masked chunks are contiguous (one strided STT op).

Layouts on device (per core):
 - xT   [1024, 4096] f32  x transposed (host-provided), streamed 512-col chunks
 - xtb  bf16 cast of the current chunk (gpsimd)
 - qTb  [128, S]  bf16  head-dim on partitions (2 heads x 64), seq on free
 - kTb  [128, S+256] bf16  (256-col zero pad on the left)
 - vv   [128, 34, 2, 66] bf16  34 key blocks of [128 keys, 64 hd + ones col]
        per head; first 2 blocks zero. Written by DMA transpose.
 - oTb  [128, S] bf16  attention output transposed (normalized in place)
 - y    [4096, 1024] f32  partial output, DMA'd from PSUM
"""

import numpy as np
import ml_dtypes

import concourse.bass as bass
import concourse.tile as tile
from concourse import bacc, mybir
from concourse.bass_utils import run_bass_kernel_spmd

# Problem constants (hardcoded per contract -- kernel.py must be self-contained)
S = 4096
D = 1024
H = 16
HD = 64
WINDOW = 256
N_CORES = 8
HPC = H // N_CORES          # heads per core = 2
DH = HPC * HD               # per-core head dims = 128
PAD = 256                   # zero left-padding of keys
SP = S + PAD                # padded key length = 4352
NEG = -1e9

F32 = mybir.dt.float32
F32R = mybir.dt.float32r
BF16 = mybir.dt.bfloat16

N_SB = S // 256             # 16 query super-blocks
N_QB = S // 128             # 32 query blocks
N_T = S // 512              # 8 projection seq chunks
KC = D // 128               # 8 contraction chunks
NVB = SP // 128             # 34 key blocks (2 zero-pad + 32)

# psum score-chunk region for window chunk c: [c0, c2, c1] so that the two
# masked chunks (c0, c2) are contiguous at regions 0..1
REGION = {0: 0, 1: 2, 2: 1}

Y_BF16 = False              # write y partials as bf16 (halves output DMA)


def _make_masks():
    """Additive masks in TRANSPOSED layout [key-in-chunk j, query-in-block qi].

    For query block at row base i0 (rows i0+qi), window chunk c covers keys
    j_global = i0 - 256 + c*128 + j.  Allowed iff 0 <= i - j_global < 256:
      c0: allowed iff qi <  j   (strict upper triangle in (qi, j))
      c1: always allowed        (no mask)
      c2: allowed iff qi >= j
    For sb=0 the out-of-range chunks are fully masked (kTb zero padding would
    otherwise contribute exp(0)=1).
    """
    j = np.arange(128)[:, None]
    qi = np.arange(128)[None, :]
    tri0 = np.where(qi < j, 0.0, NEG).astype(np.float32)
    tri2 = np.where(qi >= j, 0.0, NEG).astype(np.float32)
    negf = np.full((128, 128), NEG, dtype=np.float32)
    # region layout [c0, c2] (and [c0, c2, c1] for the sb0/u0 full mask)
    m_std = np.stack([tri0, tri2], axis=1)            # [128, 2, 128]
    m_u1 = np.stack([negf, tri2], axis=1)             # [128, 2, 128]
    m_u0 = np.stack([negf, tri2, negf], axis=1)       # [128, 3, 128]
    return m_std, m_u1, m_u0


def build_kernel(debug=False):
    nc = bacc.Bacc()

    xT = nc.dram_tensor("xT", [D, S], BF16, kind="ExternalInput")
    wq = nc.dram_tensor("wq", [D, DH], BF16, kind="ExternalInput")
    wk = nc.dram_tensor("wk", [D, DH], BF16, kind="ExternalInput")
    wv = nc.dram_tensor("wv", [D, DH], BF16, kind="ExternalInput")
    bq = nc.dram_tensor("bq", [DH], F32, kind="ExternalInput")
    wo = nc.dram_tensor("wo", [DH, D], BF16, kind="ExternalInput")
    y = nc.dram_tensor("y", [S, D], BF16 if Y_BF16 else F32,
                       kind="ExternalOutput")

    m_std, m_u1, m_u0 = _make_masks()
    m_std_d = nc.inline_tensor(m_std, name="m_std")
    m_u1_d = nc.inline_tensor(m_u1, name="m_u1")
    m_u0_d = nc.inline_tensor(m_u0, name="m_u0")
    ident_d = nc.inline_tensor(np.eye(128, dtype=np.float32), name="identf")

    scale = 1.0 / float(np.sqrt(HD))

    with tile.TileContext(nc) as tc:
        with (
            tc.tile_pool(name="consts", bufs=1) as consts,
            tc.tile_pool(name="persist", bufs=1) as persist,
            tc.tile_pool(name="xbstream", bufs=2) as xbstream,
            tc.tile_pool(name="vbpool", bufs=2) as vbpool,
            tc.tile_pool(name="ppool", bufs=3) as ppool,
            tc.tile_pool(name="rpool", bufs=2) as rpool,
            tc.tile_pool(name="proj_ps", bufs=2, space="PSUM") as proj_ps,
            tc.tile_pool(name="s_ps", bufs=3, space="PSUM") as s_ps,
            tc.tile_pool(name="ot_ps", bufs=1, space="PSUM") as ot_ps,
            tc.tile_pool(name="vt_ps", bufs=2, space="PSUM") as vt_ps,
        ):
            # ---- constants to SBUF ----
            wq_t = consts.tile([128, KC, DH], BF16, name="wq_t")
            wk_t = consts.tile([128, KC, DH], BF16, name="wk_t")
            wv_t = consts.tile([128, KC, DH], BF16, name="wv_t")
            for (t, d) in ((wq_t, wq), (wk_t, wk), (wv_t, wv)):
                d3 = d.ap().rearrange("(c p) m -> p c m", p=128)
                for c in range(KC):
                    nc.sync.dma_start(t[:, c], d3[:, c])
            wo_t = consts.tile([DH, D], BF16, name="wo_t")
            nc.sync.dma_start(wo_t, wo.ap())

            mstd = consts.tile([128, 2, 128], F32, name="mstd")
            nc.sync.dma_start(mstd, m_std_d.ap())
            mu1 = consts.tile([128, 2, 128], F32, name="mu1")
            nc.sync.dma_start(mu1, m_u1_d.ap())
            mu0 = consts.tile([128, 3, 128], F32, name="mu0")
            nc.sync.dma_start(mu0, m_u0_d.ap())

            bq_t = consts.tile([DH, 1], F32, name="bq_t")
            nc.sync.dma_start(bq_t, bq.ap().rearrange("(p o) -> p o", o=1))
            bqs = consts.tile([DH, 1], F32, name="bqs")
            nc.vector.tensor_scalar_mul(bqs, bq_t, scale)

            identf = consts.tile([128, 128], F32R, name="identf")
            nc.sync.dma_start(identf, ident_d.ap().bitcast(F32R))

            # ---- persistent activations ----
            qTb = persist.tile([128, S], BF16, name="qTb")
            kTb = persist.tile([128, SP], BF16, name="kTb")
            vv = persist.tile([128, NVB, 2, 66], BF16, name="vv")
            oTb = persist.tile([128, S], BF16, name="oTb")
            rcb0 = persist.tile([128, 256], F32, name="rcb0")
            rcb1 = persist.tile([128, 256], F32, name="rcb1")
            nc.vector.memset(kTb[:, 0:PAD], 0.0)
            nc.vector.memset(vv[:, 0:2], 0.0)          # zero pad key blocks
            nc.vector.memset(vv[:, :, :, 64:65], 1.0)  # ones col (row sums)

            def dma_x(t):
                sl = slice(t * 512, (t + 1) * 512)
                xtb = xbstream.tile([128, KC, 512], BF16, name="xtb", tag="xtb")
                for c in range(KC):
                    nc.sync.dma_start(xtb[:, c],
                                      xT.ap()[c * 128:(c + 1) * 128, sl])
                return xtb

            def proj(t, xtb):
                sl = slice(t * 512, (t + 1) * 512)
                qps = proj_ps.tile([128, 512], F32, name="qps", tag="pps")
                for c in range(KC):
                    nc.tensor.matmul(qps, wq_t[:, c], xtb[:, c],
                                     start=(c == 0), stop=(c == KC - 1))
                nc.scalar.activation(qTb[:, sl], qps,
                                     mybir.ActivationFunctionType.Identity,
                                     bias=bqs, scale=scale)

                kps = proj_ps.tile([128, 512], F32, name="kps", tag="pps")
                for c in range(KC):
                    nc.tensor.matmul(kps, wk_t[:, c], xtb[:, c],
                                     start=(c == 0), stop=(c == KC - 1))
                nc.scalar.copy(kTb[:, PAD + t * 512:PAD + (t + 1) * 512], kps)

                vps = proj_ps.tile([128, 512], F32, name="vps", tag="pps")
                for c in range(KC):
                    nc.tensor.matmul(vps, wv_t[:, c], xtb[:, c],
                                     start=(c == 0), stop=(c == KC - 1))
                vb = vbpool.tile([128, 512], F32R, name="vb", tag="vb")
                nc.scalar.copy(vb, vps)
                # [hd2, seq] -> [seq, hd] blocks via PE transpose (f32r)
                for b in range(4):
                    blk = 2 + t * 4 + b
                    vt = vt_ps.tile([128, 2, 64], F32R, name="vt", tag="vt")
                    nc.tensor.transpose(vt, vb[:, b * 128:(b + 1) * 128],
                                        identf)
                    nc.scalar.copy(vv[:, blk, :, 0:64], vt.bitcast(F32))

            def attn_core(sb):
                ot = ot_ps.tile([65, 2, 2, 128], F32, name="ot", tag="ot")
                sps_l, p_l = {}, {}
                it = [(u, h) for u in range(2) for h in range(2)]

                def scores(u, h):
                    qb = sb * 2 + u
                    qsl = slice(qb * 128, (qb + 1) * 128)
                    hsl = slice(h * 64, (h + 1) * 64)
                    wbase = qb * 128
                    sps = s_ps.tile([128, 4, 128], F32, name="sps", tag="aps")
                    for c in range(3):
                        ksl = slice(wbase + c * 128, wbase + (c + 1) * 128)
                        nc.tensor.matmul(sps[:, REGION[c]], kTb[hsl, ksl],
                                         qTb[hsl, qsl], start=True, stop=True)
                    sps_l[(u, h)] = sps

                def softmax(u, h):
                    sps = sps_l[(u, h)]
                    if sb == 0 and u == 0:
                        nc.vector.scalar_tensor_tensor(
                            sps[:, 0:3], sps[:, 0:3], 1.0, mu0,
                            op0=mybir.AluOpType.mult, op1=mybir.AluOpType.add)
                    else:
                        m = mu1 if (sb == 0 and u == 1) else mstd
                        nc.vector.scalar_tensor_tensor(
                            sps[:, 0:2], sps[:, 0:2], 1.0, m,
                            op0=mybir.AluOpType.mult, op1=mybir.AluOpType.add)
                    p = ppool.tile([128, 3, 128], BF16, name="p", tag="p")
                    nc.scalar.activation(p, sps[:, 0:3],
                                         mybir.ActivationFunctionType.Exp)
                    p_l[(u, h)] = p

                def pv(u, h):
                    qb = sb * 2 + u
                    p = p_l[(u, h)]
                    for c in range(3):
                        nc.tensor.matmul(ot[0:65, u, h, :],
                                         vv[:, qb + c, h, 0:65],
                                         p[:, REGION[c]],
                                         start=(c == 0), stop=(c == 2))

                # software-pipelined emission: keep PE fed while softmax runs
                scores(*it[0])
                softmax(*it[0])
                scores(*it[1])
                softmax(*it[1])
                pv(*it[0])
                scores(*it[2])
                softmax(*it[2])
                pv(*it[1])
                scores(*it[3])
                softmax(*it[3])
                pv(*it[2])
                pv(*it[3])

                # evacuate o (unnormalized) and the row sums
                ssl = slice(sb * 256, (sb + 1) * 256)
                for h in range(2):
                    nc.scalar.copy(oTb[h * 64:(h + 1) * 64, ssl],
                                   ot[0:64, :, h, :])
                rcq = rpool.tile([1, 2, 256], F32, name="rcq", tag="rcq")
                for h in range(2):
                    nc.vector.reciprocal(rcq[0:1, h], ot[64:65, :, h, :])
                for h, rcbh in ((0, rcb0), (1, rcb1)):
                    nc.gpsimd.partition_broadcast(rcbh, rcq[0:1, h],
                                                  channels=128)
                    hsl = slice(h * 64, (h + 1) * 64)
                    nc.vector.tensor_mul(oTb[hsl, ssl], oTb[hsl, ssl],
                                         rcbh[hsl, :])

            def y_phase(sb):
                for u in range(2):
                    qb = sb * 2 + u
                    qsl = slice(qb * 128, (qb + 1) * 128)
                    for nch in range(2):
                        yp = s_ps.tile([128, 512], F32, name="yp", tag="aps")
                        nc.tensor.matmul(yp, oTb[:, qsl],
                                         wo_t[:, nch * 512:(nch + 1) * 512],
                                         start=True, stop=True)
                        # DMA cannot read PSUM: evacuate to SBUF, alternating
                        # engines to balance scalar/vector load
                        yb = rpool.tile([128, 512], BF16 if Y_BF16 else F32,
                                        name="yb", tag="yb", bufs=3)
                        if (u + nch) % 2 == 0:
                            nc.scalar.copy(yb, yp)
                        else:
                            nc.vector.tensor_copy(yb, yp)
                        nc.sync.dma_start(
                            y.ap()[qsl, nch * 512:(nch + 1) * 512], yb)

            # ---- main interleaved loop ----
            xtb = dma_x(0)
            for t in range(N_T):
                if t + 1 < N_T:
                    xtb_n = dma_x(t + 1)
                proj(t, xtb)
                if t + 1 < N_T:
                    xtb = xtb_n
                for sb in (2 * t, 2 * t + 1):
                    attn_core(sb)
                    if sb >= 1:
                        y_phase(sb - 1)
            y_phase(N_SB - 1)

            if debug:
                d_qT = nc.dram_tensor("d_qT", [128, S], BF16,
                                      kind="ExternalOutput")
                d_kT = nc.dram_tensor("d_kT", [128, SP], BF16,
                                      kind="ExternalOutput")
                d_vv = nc.dram_tensor("d_vv", [128, NVB * 2 * 66], BF16,
                                      kind="ExternalOutput")
                d_oT = nc.dram_tensor("d_oT", [128, S], BF16,
                                      kind="ExternalOutput")
                nc.sync.dma_start(d_qT.ap(), qTb)
                nc.sync.dma_start(d_kT.ap(), kTb)
                nc.sync.dma_start(d_vv.ap(),
                                  vv.rearrange("p a b c -> p (a b c)"))
                nc.sync.dma_start(d_oT.ap(), oTb)

    if not nc.is_finalized():
        nc.finalize()
    return nc


def build_kernel_debug():
    return build_kernel(debug=True)


def _make_in_maps(x, Wq, bq, Wk, Wv, Wo):
    xT = np.ascontiguousarray(
        np.asarray(x, dtype=np.float32)[0].T).astype(ml_dtypes.bfloat16)
    Wq = np.asarray(Wq, dtype=np.float32)
    Wk = np.asarray(Wk, dtype=np.float32)
    Wv = np.asarray(Wv, dtype=np.float32)
    Wo = np.asarray(Wo, dtype=np.float32)
    bq = np.asarray(bq, dtype=np.float32)
    bf = ml_dtypes.bfloat16
    in_maps = []
    for c in range(N_CORES):
        csl = slice(c * DH, (c + 1) * DH)
        in_maps.append({
            "xT": xT,
            "wq": np.ascontiguousarray(Wq[:, csl]).astype(bf),
            "wk": np.ascontiguousarray(Wk[:, csl]).astype(bf),
            "wv": np.ascontiguousarray(Wv[:, csl]).astype(bf),
            "bq": np.ascontiguousarray(bq[csl]),
            "wo": np.ascontiguousarray(Wo[csl, :]).astype(bf),
        })
    return in_maps


_NC_CACHE = None


def kernel(x, Wq, bq, Wk, bk, Wv, bv, Wo, bo, **_kw):
    global _NC_CACHE
    x = np.asarray(x, dtype=np.float32)
    B = x.shape[0]
    assert x.shape == (B, S, D) and B == 1

    in_maps = _make_in_maps(x, Wq, bq, Wk, Wv, Wo)

    if _NC_CACHE is None:
        _NC_CACHE = build_kernel()
    res = run_bass_kernel_spmd(_NC_CACHE, in_maps, core_ids=list(range(N_CORES)))

    out = np.zeros((S, D), dtype=np.float32)
    for c in range(N_CORES):
        out += np.asarray(res.results[c]["y"], dtype=np.float32)
    # host-side bias terms: bo plus the bv @ Wo constant row (see header)
    bv = np.asarray(bv, dtype=np.float32)
    bo = np.asarray(bo, dtype=np.float32)
    Wo = np.asarray(Wo, dtype=np.float32)
    out += (bv @ Wo + bo)[None, :]
    return out.reshape(1, S, D)


# revision 25
# speedup vs baseline: 1.8131x; 1.1099x over previous
"""Local (sliding-window causal) attention on 8 Trainium2 NeuronCores. v3.

Problem: B=1, S=4096, D=1024, H=16 heads (hd=64), WINDOW=256.
Sharding: tensor-parallel over heads -- 2 heads per core. Each core computes
q/k/v projections for its 2 heads, windowed softmax attention, and its
partial contribution o_c @ Wo_c. The host sums the 8 partials (bf16) and
adds the bias terms.

Design notes (evolved from trace analysis):
 - All matmuls bf16 (FWL weight loads, full PE rate at any N). x is cast to
   bf16 on the host; weights are passed bf16.
 - Scores are computed TRANSPOSED (keys on partitions): p = exp(s) feeds the
   PV matmul straight from SBUF -- no probability transposes, no PSUM->SBUF
   p copies. Row sums come free from a ones-column appended to V; softmax
   normalization = one divide on oT per head per super-block, using a
   partition-broadcast (gpsimd) of the row-sum vector.
 - Both heads share one PSUM score tile [128, 2h, 4r, 128] so the mask and
   exp run as single wide strided ops (halves per-op overhead).
 - DMA dispatch on the sync engine costs ~600ns per instruction regardless
   of size, so transfers are batched: one DMA per weight matrix (strided
   3D AP), one per x chunk, one per y super-block. y partials written bf16.
 - The per-128-query 384-key window splits into 3 aligned chunks: c0 (strict
   upper tri mask), c1 (no mask), c2 (lower incl tri). PSUM region order is
   [c0, c2, c1] so masked chunks are contiguous.
 - Projections, attention, and the output GEMM are software-pipelined so the
   PE stays busy (keeps the HAM clock gate warm).
"""

import numpy as np
import ml_dtypes

import concourse.bass as bass
import concourse.tile as tile
from concourse import bacc, mybir
from concourse.bass_utils import run_bass_kernel_spmd

# Problem constants (hardcoded per contract -- kernel.py must be self-contained)
S = 4096
D = 1024
H = 16
HD = 64
WINDOW = 256
N_CORES = 8
HPC = H // N_CORES          # heads per core = 2
DH = HPC * HD               # per-core head dims = 128
PAD = 256                   # zero left-padding of keys
SP = S + PAD                # padded key length = 4352
NEG = -1e9

F32 = mybir.dt.float32
F32R = mybir.dt.float32r
BF16 = mybir.dt.bfloat16

N_SB = S // 256             # 16 query super-blocks
N_QB = S // 128             # 32 query blocks
N_T = S // 512              # 8 projection seq chunks
KC = D // 128               # 8 contraction chunks
NVB = SP // 128             # 34 key blocks (2 zero-pad + 32)

# psum score-chunk region for window chunk c: [c0, c2, c1] so that the two
# masked chunks (c0, c2) are contiguous at regions 0..1
REGION = {0: 0, 1: 2, 2: 1}

NORM_DIV = False            # use DVE tensor_tensor divide; False -> recip+mul


def _make_masks():
    """Additive masks in TRANSPOSED layout [key-in-chunk j, query-in-block qi].

    For query block at row base i0 (rows i0+qi), window chunk c covers keys
    j_global = i0 - 256 + c*128 + j.  Allowed iff 0 <= i - j_global < 256:
      c0: allowed iff qi <  j
      c1: always allowed (no mask)
      c2: allowed iff qi >= j
    For sb=0 the out-of-range chunks are fully masked (kTb zero padding
    would otherwise contribute exp(0)=1). Masks are doubled along an h axis
    so one strided op covers both heads: [j, 2h, nreg, qi].
    """
    j = np.arange(128)[:, None]
    qi = np.arange(128)[None, :]
    tri0 = np.where(qi < j, 0.0, NEG).astype(np.float32)
    tri2 = np.where(qi >= j, 0.0, NEG).astype(np.float32)
    negf = np.full((128, 128), NEG, dtype=np.float32)
    std = np.stack([tri0, tri2], axis=1)               # [128, 2, 128]
    u1 = np.stack([negf, tri2], axis=1)
    u0 = np.stack([negf, tri2, negf], axis=1)          # [128, 3, 128]
    m_std = np.stack([std, std], axis=1)               # [128, 2, 2, 128]
    m_u1 = np.stack([u1, u1], axis=1)
    m_u0 = np.stack([u0, u0], axis=1)                  # [128, 2, 3, 128]
    # one flat const: std (2*2*128) | u1 (2*2*128) | u0 (2*3*128)
    flat = np.concatenate([m_std.reshape(128, -1), m_u1.reshape(128, -1),
                           m_u0.reshape(128, -1)], axis=1)
    return np.ascontiguousarray(flat)


def build_kernel(debug=False):
    nc = bacc.Bacc()

    xT = nc.dram_tensor("xT", [D, S], BF16, kind="ExternalInput")
    wq = nc.dram_tensor("wq", [D, DH], BF16, kind="ExternalInput")
    wk = nc.dram_tensor("wk", [D, DH], BF16, kind="ExternalInput")
    wv = nc.dram_tensor("wv", [D, DH], BF16, kind="ExternalInput")
    bq = nc.dram_tensor("bq", [DH], F32, kind="ExternalInput")
    wo = nc.dram_tensor("wo", [DH, D], BF16, kind="ExternalInput")
    y = nc.dram_tensor("y", [S, D], BF16, kind="ExternalOutput")

    mask_flat = _make_masks()
    mask_d = nc.inline_tensor(mask_flat, name="mask_all")
    ident_d = nc.inline_tensor(np.eye(128, dtype=np.float32), name="identf")

    scale = 1.0 / float(np.sqrt(HD))

    with tile.TileContext(nc) as tc:
        with (
            tc.tile_pool(name="consts", bufs=1) as consts,
            tc.tile_pool(name="persist", bufs=1) as persist,
            tc.tile_pool(name="xbstream", bufs=2) as xbstream,
            tc.tile_pool(name="vbpool", bufs=2) as vbpool,
            tc.tile_pool(name="ppool", bufs=3) as ppool,
            tc.tile_pool(name="rpool", bufs=2) as rpool,
            tc.tile_pool(name="ybpool", bufs=2) as ybpool,
            tc.tile_pool(name="proj_ps", bufs=2, space="PSUM") as proj_ps,
            tc.tile_pool(name="s_ps", bufs=2, space="PSUM") as s_ps,
            tc.tile_pool(name="ot_ps", bufs=1, space="PSUM") as ot_ps,
            tc.tile_pool(name="vt_ps", bufs=1, space="PSUM") as vt_ps,
        ):
            # ---- constants to SBUF (batched DMAs) ----
            wq_t = consts.tile([128, KC, DH], BF16, name="wq_t")
            wk_t = consts.tile([128, KC, DH], BF16, name="wk_t")
            wv_t = consts.tile([128, KC, DH], BF16, name="wv_t")
            for (t, d) in ((wq_t, wq), (wk_t, wk), (wv_t, wv)):
                nc.sync.dma_start(t, d.ap().rearrange("(c p) m -> p c m",
                                                      p=128))
            wo_t = consts.tile([DH, D], BF16, name="wo_t")
            nc.sync.dma_start(wo_t, wo.ap())

            masks = consts.tile([128, 14, 128], F32, name="masks")
            nc.sync.dma_start(masks, mask_d.ap())
            mstd = masks[:, 0:4].rearrange("p (h r) q -> p h r q", h=2)
            mu1 = masks[:, 4:8].rearrange("p (h r) q -> p h r q", h=2)
            mu0 = masks[:, 8:14].rearrange("p (h r) q -> p h r q", h=2)

            bq_t = consts.tile([DH, 1], F32, name="bq_t")
            nc.sync.dma_start(bq_t, bq.ap().rearrange("(p o) -> p o", o=1))
            bqs = consts.tile([DH, 1], F32, name="bqs")
            nc.vector.tensor_scalar_mul(bqs, bq_t, scale)

            identf = consts.tile([128, 128], F32R, name="identf")
            nc.sync.dma_start(identf, ident_d.ap().bitcast(F32R))

            # ---- persistent activations ----
            qTb = persist.tile([128, S], BF16, name="qTb")
            kTb = persist.tile([128, SP], BF16, name="kTb")
            vv = persist.tile([128, NVB, 2, 66], BF16, name="vv")
            oTb = persist.tile([128, S], BF16, name="oTb")
            rsb0 = persist.tile([128, 256], F32, name="rsb0")
            rsb1 = persist.tile([128, 256], F32, name="rsb1")
            nc.vector.memset(kTb[:, 0:PAD], 0.0)
            nc.vector.memset(vv[:, 0:2], 0.0)          # zero pad key blocks
            nc.vector.memset(vv[:, :, :, 64:65], 1.0)  # ones col (row sums)

            def dma_x(t):
                xtb = xbstream.tile([128, KC, 512], BF16, name="xtb",
                                    tag="xtb")
                src = xT.ap().rearrange("(c p) s -> p c s", p=128)
                nc.sync.dma_start(xtb, src[:, :, t * 512:(t + 1) * 512])
                return xtb

            def proj(t, xtb):
                sl = slice(t * 512, (t + 1) * 512)
                qps = proj_ps.tile([128, 512], F32, name="qps", tag="pps")
                for c in range(KC):
                    nc.tensor.matmul(qps, wq_t[:, c], xtb[:, c],
                                     start=(c == 0), stop=(c == KC - 1))
                nc.scalar.activation(qTb[:, sl], qps,
                                     mybir.ActivationFunctionType.Identity,
                                     bias=bqs, scale=scale)

                kps = proj_ps.tile([128, 512], F32, name="kps", tag="pps")
                for c in range(KC):
                    nc.tensor.matmul(kps, wk_t[:, c], xtb[:, c],
                                     start=(c == 0), stop=(c == KC - 1))
                nc.scalar.copy(kTb[:, PAD + t * 512:PAD + (t + 1) * 512], kps)

                vps = proj_ps.tile([128, 512], F32, name="vps", tag="pps")
                for c in range(KC):
                    nc.tensor.matmul(vps, wv_t[:, c], xtb[:, c],
                                     start=(c == 0), stop=(c == KC - 1))
                vb = vbpool.tile([128, 512], F32R, name="vb", tag="vb")
                nc.scalar.copy(vb, vps)
                # [hd2, seq] -> [seq, hd] blocks via PE transpose (f32r)
                for b in range(4):
                    blk = 2 + t * 4 + b
                    vt = vt_ps.tile([128, 2, 64], F32R, name="vt", tag="vt")
                    nc.tensor.transpose(vt, vb[:, b * 128:(b + 1) * 128],
                                        identf)
                    nc.scalar.copy(vv[:, blk, :, 0:64], vt.bitcast(F32))

            def attn_core(sb):
                ot = ot_ps.tile([65, 2, 2, 128], F32, name="ot", tag="ot")
                sps_l, p_l = {}, {}

                def scores(u):
                    qb = sb * 2 + u
                    qsl = slice(qb * 128, (qb + 1) * 128)
                    wbase = qb * 128
                    sps = s_ps.tile([128, 2, 4, 128], F32, name="sps",
                                    tag="aps")
                    for h in range(2):
                        hsl = slice(h * 64, (h + 1) * 64)
                        for c in range(3):
                            ksl = slice(wbase + c * 128,
                                        wbase + (c + 1) * 128)
                            nc.tensor.matmul(sps[:, h, REGION[c]],
                                             kTb[hsl, ksl], qTb[hsl, qsl],
                                             start=True, stop=True)
                    sps_l[u] = sps

                def softmax(u):
                    sps = sps_l[u]
                    if sb == 0 and u == 0:
                        nc.vector.scalar_tensor_tensor(
                            sps[:, :, 0:3], sps[:, :, 0:3], 1.0, mu0,
                            op0=mybir.AluOpType.mult, op1=mybir.AluOpType.add)
                    else:
                        m = mu1 if (sb == 0 and u == 1) else mstd
                        nc.vector.scalar_tensor_tensor(
                            sps[:, :, 0:2], sps[:, :, 0:2], 1.0, m,
                            op0=mybir.AluOpType.mult, op1=mybir.AluOpType.add)
                    p = ppool.tile([128, 2, 3, 128], BF16, name="p", tag="p")
                    nc.scalar.activation(p, sps[:, :, 0:3],
                                         mybir.ActivationFunctionType.Exp)
                    p_l[u] = p

                def pv(u):
                    qb = sb * 2 + u
                    p = p_l[u]
                    for h in range(2):
                        for c in range(3):
                            nc.tensor.matmul(ot[0:65, u, h, :],
                                             vv[:, qb + c, h, 0:65],
                                             p[:, h, REGION[c]],
                                             start=(c == 0), stop=(c == 2))

                # software-pipelined emission: keep PE fed while softmax runs
                scores(0)
                softmax(0)
                scores(1)
                softmax(1)
                pv(0)
                pv(1)

                # evacuate o (unnormalized) and the row sums; then normalize
                # oT in place: oT[h] /= rowsum[h] (broadcast across hd)
                ssl = slice(sb * 256, (sb + 1) * 256)
                rsq = rpool.tile([1, 2, 256], F32, name="rsq", tag="rsq")
                for h in range(2):
                    nc.scalar.copy(oTb[h * 64:(h + 1) * 64, ssl],
                                   ot[0:64, :, h, :])
                    nc.scalar.copy(rsq[0:1, h], ot[64:65, :, h, :])
                for h, rsb in ((0, rsb0), (1, rsb1)):
                    nc.gpsimd.partition_broadcast(rsb, rsq[0:1, h],
                                                  channels=128)
                    hsl = slice(h * 64, (h + 1) * 64)
                    if NORM_DIV:
                        nc.vector.tensor_tensor(
                            oTb[hsl, ssl], oTb[hsl, ssl], rsb[hsl, :],
                            op=mybir.AluOpType.divide)
                    else:
                        nc.vector.reciprocal(rsb, rsb)
                        nc.vector.tensor_mul(oTb[hsl, ssl], oTb[hsl, ssl],
                                             rsb[hsl, :])

            def y_phase(sb):
                ysb = ybpool.tile([128, 2, 1024], BF16, name="ysb", tag="ysb")
                for u in range(2):
                    qb = sb * 2 + u
                    qsl = slice(qb * 128, (qb + 1) * 128)
                    yp = s_ps.tile([128, 2, 4, 128], F32, name="yp",
                                   tag="aps")
                    for nch in range(2):
                        nc.tensor.matmul(yp[:, nch], oTb[:, qsl],
                                         wo_t[:, nch * 512:(nch + 1) * 512],
                                         start=True, stop=True)
                    nc.scalar.copy(ysb[:, u, 0:512], yp[:, 0])
                    nc.vector.tensor_copy(ysb[:, u, 512:1024], yp[:, 1])
                dst = y.ap()[sb * 256:(sb + 1) * 256, :].rearrange(
                    "(u p) m -> p u m", p=128)
                nc.sync.dma_start(dst, ysb)

            # ---- main interleaved loop ----
            xtb = dma_x(0)
            for t in range(N_T):
                if t + 1 < N_T:
                    xtb_n = dma_x(t + 1)
                proj(t, xtb)
                if t + 1 < N_T:
                    xtb = xtb_n
                for sb in (2 * t, 2 * t + 1):
                    attn_core(sb)
                    if sb >= 1:
                        y_phase(sb - 1)
            y_phase(N_SB - 1)

            if debug:
                d_qT = nc.dram_tensor("d_qT", [128, S], BF16,
                                      kind="ExternalOutput")
                d_kT = nc.dram_tensor("d_kT", [128, SP], BF16,
                                      kind="ExternalOutput")
                d_vv = nc.dram_tensor("d_vv", [128, NVB * 2 * 66], BF16,
                                      kind="ExternalOutput")
                d_oT = nc.dram_tensor("d_oT", [128, S], BF16,
                                      kind="ExternalOutput")
                nc.sync.dma_start(d_qT.ap(), qTb)
                nc.sync.dma_start(d_kT.ap(), kTb)
                nc.sync.dma_start(d_vv.ap(),
                                  vv.rearrange("p a b c -> p (a b c)"))
                nc.sync.dma_start(d_oT.ap(), oTb)

    if not nc.is_finalized():
        nc.finalize()
    return nc


def build_kernel_debug():
    return build_kernel(debug=True)


def _make_in_maps(x, Wq, bq, Wk, Wv, Wo):
    xT = np.ascontiguousarray(
        np.asarray(x, dtype=np.float32)[0].T).astype(ml_dtypes.bfloat16)
    Wq = np.asarray(Wq, dtype=np.float32)
    Wk = np.asarray(Wk, dtype=np.float32)
    Wv = np.asarray(Wv, dtype=np.float32)
    Wo = np.asarray(Wo, dtype=np.float32)
    bq = np.asarray(bq, dtype=np.float32)
    bf = ml_dtypes.bfloat16
    in_maps = []
    for c in range(N_CORES):
        csl = slice(c * DH, (c + 1) * DH)
        in_maps.append({
            "xT": xT,
            "wq": np.ascontiguousarray(Wq[:, csl]).astype(bf),
            "wk": np.ascontiguousarray(Wk[:, csl]).astype(bf),
            "wv": np.ascontiguousarray(Wv[:, csl]).astype(bf),
            "bq": np.ascontiguousarray(bq[csl]),
            "wo": np.ascontiguousarray(Wo[csl, :]).astype(bf),
        })
    return in_maps


_NC_CACHE = None


def kernel(x, Wq, bq, Wk, bk, Wv, bv, Wo, bo, **_kw):
    global _NC_CACHE
    x = np.asarray(x, dtype=np.float32)
    B = x.shape[0]
    assert x.shape == (B, S, D) and B == 1

    in_maps = _make_in_maps(x, Wq, bq, Wk, Wv, Wo)

    if _NC_CACHE is None:
        _NC_CACHE = build_kernel()
    res = run_bass_kernel_spmd(_NC_CACHE, in_maps, core_ids=list(range(N_CORES)))

    out = np.zeros((S, D), dtype=np.float32)
    for c in range(N_CORES):
        out += np.asarray(res.results[c]["y"], dtype=np.float32)
    # host-side bias terms: bo plus the bv @ Wo constant row (see header)
    bv = np.asarray(bv, dtype=np.float32)
    bo = np.asarray(bo, dtype=np.float32)
    Wo = np.asarray(Wo, dtype=np.float32)
    out += (bv @ Wo + bo)[None, :]
    return out.reshape(1, S, D)


# revision 26
# speedup vs baseline: 1.9984x; 1.1022x over previous
"""Local (sliding-window causal) attention on 8 Trainium2 NeuronCores. v3.

Problem: B=1, S=4096, D=1024, H=16 heads (hd=64), WINDOW=256.
Sharding: tensor-parallel over heads -- 2 heads per core. Each core computes
q/k/v projections for its 2 heads, windowed softmax attention, and its
partial contribution o_c @ Wo_c. The host sums the 8 partials (bf16) and
adds the bias terms.

Design notes (evolved from trace analysis):
 - All matmuls bf16 (FWL weight loads, full PE rate at any N). x is cast to
   bf16 on the host; weights are passed bf16.
 - Scores are computed TRANSPOSED (keys on partitions): p = exp(s) feeds the
   PV matmul straight from SBUF -- no probability transposes, no PSUM->SBUF
   p copies. Row sums come free from a ones-column appended to V; softmax
   normalization = one divide on oT per head per super-block, using a
   partition-broadcast (gpsimd) of the row-sum vector.
 - Both heads share one PSUM score tile [128, 2h, 4r, 128] so the mask and
   exp run as single wide strided ops (halves per-op overhead).
 - DMA dispatch on the sync engine costs ~600ns per instruction regardless
   of size, so transfers are batched: one DMA per weight matrix (strided
   3D AP), one per x chunk, one per y super-block. y partials written bf16.
 - The per-128-query 384-key window splits into 3 aligned chunks: c0 (strict
   upper tri mask), c1 (no mask), c2 (lower incl tri). PSUM region order is
   [c0, c2, c1] so masked chunks are contiguous.
 - Projections, attention, and the output GEMM are software-pipelined so the
   PE stays busy (keeps the HAM clock gate warm).
"""

import numpy as np
import ml_dtypes

import concourse.bass as bass
import concourse.tile as tile
from concourse import bacc, mybir
from concourse.bass_utils import run_bass_kernel_spmd

# Problem constants (hardcoded per contract -- kernel.py must be self-contained)
S = 4096
D = 1024
H = 16
HD = 64
WINDOW = 256
N_CORES = 8
HPC = H // N_CORES          # heads per core = 2
DH = HPC * HD               # per-core head dims = 128
PAD = 256                   # zero left-padding of keys
SP = S + PAD                # padded key length = 4352
NEG = -1e9

F32 = mybir.dt.float32
F32R = mybir.dt.float32r
BF16 = mybir.dt.bfloat16

N_SB = S // 256             # 16 query super-blocks
N_QB = S // 128             # 32 query blocks
N_T = S // 512              # 8 projection seq chunks
KC = D // 128               # 8 contraction chunks
NVB = SP // 128             # 34 key blocks (2 zero-pad + 32)

# psum score-chunk region for window chunk c: [c0, c2, c1] so that the two
# masked chunks (c0, c2) are contiguous at regions 0..1
REGION = {0: 0, 1: 2, 2: 1}

NORM_DIV = False            # use DVE tensor_tensor divide; False -> recip+mul


def _make_masks():
    """Additive masks in TRANSPOSED layout [key-in-chunk j, query-in-block qi].

    For query block at row base i0 (rows i0+qi), window chunk c covers keys
    j_global = i0 - 256 + c*128 + j.  Allowed iff 0 <= i - j_global < 256:
      c0: allowed iff qi <  j
      c1: always allowed (no mask)
      c2: allowed iff qi >= j
    For sb=0 the out-of-range chunks are fully masked (kTb zero padding
    would otherwise contribute exp(0)=1). Masks are doubled along an h axis
    so one strided op covers both heads: [j, 2h, nreg, qi].
    """
    j = np.arange(128)[:, None]
    qi = np.arange(128)[None, :]
    tri0 = np.where(qi < j, 0.0, NEG).astype(np.float32)
    tri2 = np.where(qi >= j, 0.0, NEG).astype(np.float32)
    negf = np.full((128, 128), NEG, dtype=np.float32)
    std = np.stack([tri0, tri2], axis=1)               # [128, 2, 128]
    u1 = np.stack([negf, tri2], axis=1)
    u0 = np.stack([negf, tri2, negf], axis=1)          # [128, 3, 128]
    m_std = np.stack([std, std], axis=1)               # [128, 2, 2, 128]
    m_u1 = np.stack([u1, u1], axis=1)
    m_u0 = np.stack([u0, u0], axis=1)                  # [128, 2, 3, 128]
    # one flat const: std (2*2*128) | u1 (2*2*128) | u0 (2*3*128)
    flat = np.concatenate([m_std.reshape(128, -1), m_u1.reshape(128, -1),
                           m_u0.reshape(128, -1)], axis=1)
    return np.ascontiguousarray(flat)


def build_kernel(debug=False):
    nc = bacc.Bacc()

    xT = nc.dram_tensor("xT", [D, S], BF16, kind="ExternalInput")
    wq = nc.dram_tensor("wq", [D, DH], BF16, kind="ExternalInput")
    wk = nc.dram_tensor("wk", [D, DH], BF16, kind="ExternalInput")
    wv = nc.dram_tensor("wv", [D, DH], BF16, kind="ExternalInput")
    bq = nc.dram_tensor("bq", [DH], F32, kind="ExternalInput")
    wo = nc.dram_tensor("wo", [DH, D], BF16, kind="ExternalInput")
    y = nc.dram_tensor("y", [S, D], BF16, kind="ExternalOutput")

    mask_flat = _make_masks()
    mask_d = nc.inline_tensor(mask_flat, name="mask_all")
    ident_d = nc.inline_tensor(np.eye(128, dtype=np.float32), name="identf")

    scale = 1.0 / float(np.sqrt(HD))

    with tile.TileContext(nc) as tc:
        with (
            tc.tile_pool(name="consts", bufs=1) as consts,
            tc.tile_pool(name="persist", bufs=1) as persist,
            tc.tile_pool(name="xbstream", bufs=2) as xbstream,
            tc.tile_pool(name="vbpool", bufs=2) as vbpool,
            tc.tile_pool(name="ppool", bufs=3) as ppool,
            tc.tile_pool(name="rpool", bufs=2) as rpool,
            tc.tile_pool(name="ybpool", bufs=2) as ybpool,
            tc.tile_pool(name="proj_ps", bufs=2, space="PSUM") as proj_ps,
            tc.tile_pool(name="s_ps", bufs=2, space="PSUM") as s_ps,
            tc.tile_pool(name="ot_ps", bufs=2, space="PSUM") as ot_ps,
        ):
            # ---- constants to SBUF (batched DMAs) ----
            wq_t = consts.tile([128, KC, DH], BF16, name="wq_t")
            wk_t = consts.tile([128, KC, DH], BF16, name="wk_t")
            wv_t = consts.tile([128, KC, DH], BF16, name="wv_t")
            for (t, d) in ((wq_t, wq), (wk_t, wk), (wv_t, wv)):
                nc.sync.dma_start(t, d.ap().rearrange("(c p) m -> p c m",
                                                      p=128))
            wo_t = consts.tile([DH, D], BF16, name="wo_t")
            nc.sync.dma_start(wo_t, wo.ap())

            masks = consts.tile([128, 14, 128], F32, name="masks")
            nc.sync.dma_start(masks, mask_d.ap())
            mstd = masks[:, 0:4].rearrange("p (h r) q -> p h r q", h=2)
            mu1 = masks[:, 4:8].rearrange("p (h r) q -> p h r q", h=2)
            mu0 = masks[:, 8:14].rearrange("p (h r) q -> p h r q", h=2)

            bq_t = consts.tile([DH, 1], F32, name="bq_t")
            nc.sync.dma_start(bq_t, bq.ap().rearrange("(p o) -> p o", o=1))
            bqs = consts.tile([DH, 1], F32, name="bqs")
            nc.vector.tensor_scalar_mul(bqs, bq_t, scale)

            identf = consts.tile([128, 128], F32R, name="identf")
            nc.sync.dma_start(identf, ident_d.ap().bitcast(F32R))

            # ---- persistent activations ----
            qTb = persist.tile([128, S], BF16, name="qTb")
            kTb = persist.tile([128, SP], BF16, name="kTb")
            vv = persist.tile([128, NVB, 2, 66], BF16, name="vv")
            oTb = persist.tile([128, S], BF16, name="oTb")
            rsb0 = persist.tile([128, 256], F32, name="rsb0")
            rsb1 = persist.tile([128, 256], F32, name="rsb1")
            nc.vector.memset(kTb[:, 0:PAD], 0.0)
            nc.vector.memset(vv[:, 0:2], 0.0)          # zero pad key blocks
            nc.vector.memset(vv[:, :, :, 64:65], 1.0)  # ones col (row sums)

            def dma_x(t):
                xtb = xbstream.tile([128, KC, 512], BF16, name="xtb",
                                    tag="xtb")
                src = xT.ap().rearrange("(c p) s -> p c s", p=128)
                nc.sync.dma_start(xtb, src[:, :, t * 512:(t + 1) * 512])
                return xtb

            def proj(t, xtb):
                sl = slice(t * 512, (t + 1) * 512)
                qps = proj_ps.tile([128, 512], F32, name="qps", tag="pps")
                for c in range(KC):
                    nc.tensor.matmul(qps, wq_t[:, c], xtb[:, c],
                                     start=(c == 0), stop=(c == KC - 1))
                nc.scalar.activation(qTb[:, sl], qps,
                                     mybir.ActivationFunctionType.Identity,
                                     bias=bqs, scale=scale)

                kps = proj_ps.tile([128, 512], F32, name="kps", tag="pps")
                for c in range(KC):
                    nc.tensor.matmul(kps, wk_t[:, c], xtb[:, c],
                                     start=(c == 0), stop=(c == KC - 1))
                nc.scalar.copy(kTb[:, PAD + t * 512:PAD + (t + 1) * 512], kps)

                vps = proj_ps.tile([128, 512], F32, name="vps", tag="pps")
                for c in range(KC):
                    nc.tensor.matmul(vps, wv_t[:, c], xtb[:, c],
                                     start=(c == 0), stop=(c == KC - 1))
                vb = vbpool.tile([128, 512], F32R, name="vb", tag="vb")
                nc.scalar.copy(vb, vps)
                # [hd2, seq] -> [seq, hd] blocks via PE transpose (f32r)
                for b in range(4):
                    blk = 2 + t * 4 + b
                    vt = proj_ps.tile([128, 2, 64], F32R, name="vt",
                                      tag="pps")
                    nc.tensor.transpose(vt, vb[:, b * 128:(b + 1) * 128],
                                        identf)
                    nc.scalar.copy(vv[:, blk, :, 0:64], vt.bitcast(F32))

            def attn_core(sb):
                ot = ot_ps.tile([65, 2, 2, 128], F32, name="ot", tag="ot")
                sps_l, p_l = {}, {}

                def scores(u):
                    qb = sb * 2 + u
                    qsl = slice(qb * 128, (qb + 1) * 128)
                    wbase = qb * 128
                    sps = s_ps.tile([128, 2, 4, 128], F32, name="sps",
                                    tag="aps")
                    for h in range(2):
                        hsl = slice(h * 64, (h + 1) * 64)
                        for c in range(3):
                            ksl = slice(wbase + c * 128,
                                        wbase + (c + 1) * 128)
                            nc.tensor.matmul(sps[:, h, REGION[c]],
                                             kTb[hsl, ksl], qTb[hsl, qsl],
                                             start=True, stop=True)
                    sps_l[u] = sps

                def softmax(u):
                    sps = sps_l[u]
                    if sb == 0 and u == 0:
                        nc.vector.scalar_tensor_tensor(
                            sps[:, :, 0:3], sps[:, :, 0:3], 1.0, mu0,
                            op0=mybir.AluOpType.mult, op1=mybir.AluOpType.add)
                    else:
                        m = mu1 if (sb == 0 and u == 1) else mstd
                        nc.vector.scalar_tensor_tensor(
                            sps[:, :, 0:2], sps[:, :, 0:2], 1.0, m,
                            op0=mybir.AluOpType.mult, op1=mybir.AluOpType.add)
                    p = ppool.tile([128, 2, 3, 128], BF16, name="p", tag="p")
                    nc.scalar.activation(p, sps[:, :, 0:3],
                                         mybir.ActivationFunctionType.Exp)
                    p_l[u] = p

                def pv(u):
                    qb = sb * 2 + u
                    p = p_l[u]
                    for h in range(2):
                        for c in range(3):
                            nc.tensor.matmul(ot[0:65, u, h, :],
                                             vv[:, qb + c, h, 0:65],
                                             p[:, h, REGION[c]],
                                             start=(c == 0), stop=(c == 2))

                # software-pipelined emission: keep PE fed while softmax runs
                scores(0)
                softmax(0)
                scores(1)
                softmax(1)
                pv(0)
                pv(1)

                # evacuate o (unnormalized) and the row sums; then normalize
                # oT in place: oT[h] /= rowsum[h] (broadcast across hd)
                ssl = slice(sb * 256, (sb + 1) * 256)
                rsq = rpool.tile([1, 2, 2, 128], F32, name="rsq", tag="rsq")
                # one copy, free-dim order permuted (u,h,q) -> (h,u,q)
                nc.scalar.copy(rsq.rearrange("p h u q -> p u h q"),
                               ot[64:65, :, :, :])
                nc.scalar.copy(oTb[0:64, ssl], ot[0:64, :, 0, :])
                nc.vector.tensor_copy(oTb[64:128, ssl], ot[0:64, :, 1, :])
                for h, rsb in ((0, rsb0), (1, rsb1)):
                    nc.gpsimd.partition_broadcast(rsb, rsq[0:1, h],
                                                  channels=128)
                    hsl = slice(h * 64, (h + 1) * 64)
                    nc.vector.reciprocal_approx_fast(rsb, rsb)
                    nc.vector.tensor_mul(oTb[hsl, ssl], oTb[hsl, ssl],
                                         rsb[hsl, :])

            def y_phase(sb):
                ysb = ybpool.tile([128, 2, 1024], BF16, name="ysb", tag="ysb")
                for u in range(2):
                    qb = sb * 2 + u
                    qsl = slice(qb * 128, (qb + 1) * 128)
                    yp = s_ps.tile([128, 2, 4, 128], F32, name="yp",
                                   tag="aps")
                    for nch in range(2):
                        nc.tensor.matmul(yp[:, nch], oTb[:, qsl],
                                         wo_t[:, nch * 512:(nch + 1) * 512],
                                         start=True, stop=True)
                    nc.scalar.copy(ysb[:, u, 0:512], yp[:, 0])
                    nc.vector.tensor_copy(ysb[:, u, 512:1024], yp[:, 1])
                dst = y.ap()[sb * 256:(sb + 1) * 256, :].rearrange(
                    "(u p) m -> p u m", p=128)
                nc.sync.dma_start(dst, ysb)

            # ---- main interleaved loop ----
            xtb = dma_x(0)
            for t in range(N_T):
                if t + 1 < N_T:
                    xtb_n = dma_x(t + 1)
                proj(t, xtb)
                if t + 1 < N_T:
                    xtb = xtb_n
                for sb in (2 * t, 2 * t + 1):
                    attn_core(sb)
                    if sb >= 1:
                        y_phase(sb - 1)
            y_phase(N_SB - 1)

            if debug:
                d_qT = nc.dram_tensor("d_qT", [128, S], BF16,
                                      kind="ExternalOutput")
                d_kT = nc.dram_tensor("d_kT", [128, SP], BF16,
                                      kind="ExternalOutput")
                d_vv = nc.dram_tensor("d_vv", [128, NVB * 2 * 66], BF16,
                                      kind="ExternalOutput")
                d_oT = nc.dram_tensor("d_oT", [128, S], BF16,
                                      kind="ExternalOutput")
                nc.sync.dma_start(d_qT.ap(), qTb)
                nc.sync.dma_start(d_kT.ap(), kTb)
                nc.sync.dma_start(d_vv.ap(),
                                  vv.rearrange("p a b c -> p (a b c)"))
                nc.sync.dma_start(d_oT.ap(), oTb)

    if not nc.is_finalized():
        nc.finalize()
    return nc


def build_kernel_debug():
    return build_kernel(debug=True)


def _make_in_maps(x, Wq, bq, Wk, Wv, Wo):
    xT = np.ascontiguousarray(
        np.asarray(x, dtype=np.float32)[0].T).astype(ml_dtypes.bfloat16)
    Wq = np.asarray(Wq, dtype=np.float32)
    Wk = np.asarray(Wk, dtype=np.float32)
    Wv = np.asarray(Wv, dtype=np.float32)
    Wo = np.asarray(Wo, dtype=np.float32)
    bq = np.asarray(bq, dtype=np.float32)
    bf = ml_dtypes.bfloat16
    in_maps = []
    for c in range(N_CORES):
        csl = slice(c * DH, (c + 1) * DH)
        in_maps.append({
            "xT": xT,
            "wq": np.ascontiguousarray(Wq[:, csl]).astype(bf),
            "wk": np.ascontiguousarray(Wk[:, csl]).astype(bf),
            "wv": np.ascontiguousarray(Wv[:, csl]).astype(bf),
            "bq": np.ascontiguousarray(bq[csl]),
            "wo": np.ascontiguousarray(Wo[csl, :]).astype(bf),
        })
    return in_maps


_NC_CACHE = None


def kernel(x, Wq, bq, Wk, bk, Wv, bv, Wo, bo, **_kw):
    global _NC_CACHE
    x = np.asarray(x, dtype=np.float32)
    B = x.shape[0]
    assert x.shape == (B, S, D) and B == 1

    in_maps = _make_in_maps(x, Wq, bq, Wk, Wv, Wo)

    if _NC_CACHE is None:
        _NC_CACHE = build_kernel()
    res = run_bass_kernel_spmd(_NC_CACHE, in_maps, core_ids=list(range(N_CORES)))

    out = np.zeros((S, D), dtype=np.float32)
    for c in range(N_CORES):
        out += np.asarray(res.results[c]["y"], dtype=np.float32)
    # host-side bias terms: bo plus the bv @ Wo constant row (see header)
    bv = np.asarray(bv, dtype=np.float32)
    bo = np.asarray(bo, dtype=np.float32)
    Wo = np.asarray(Wo, dtype=np.float32)
    out += (bv @ Wo + bo)[None, :]
    return out.reshape(1, S, D)
